# revision 2
# baseline (speedup 1.0000x reference)
"""Trainium2 Bass kernel for nn_Attention_7945689497706.

Distribution: data-parallel over batch, 2 batch elements per core, weights
replicated, no collectives.

Per-core layout (v2):
  - RMSNorm via ones-matmul partition reduction; rsqrt computed as
    exp(-0.5*ln(ss/C)) so the whole kernel stays in one ACT table set.
  - Attention transposed (j on psum partitions), processed per head-pair
    (heads 2P, 2P+1). qT packs the pair on partitions (A rows 0:64,
    B rows 64:128); the two K=64 sim matmuls of a chunk run concurrently
    as row-tiled tiles (0,0)/(64,0) writing the two halves of one
    [128,1024] psum tile, so one exp covers both heads.
  - av lhsT = [v | ones] (65 cols): the ones column accumulates softmax
    denominators in psum row 64. Normalization reads av psum directly:
    denom row -> DVE fast-reciprocal [1,512] -> gpsimd partition_broadcast
    -> DVE multiply (psum x sbuf) into the attn tile.
  - mem_kv sims for 4 heads packed in one [128,1024] psum tile (head h at
    rows 32*(h%4), M=32 zero-padded weights), one exp per tile; mem-av
    contributions are K=4 row-tiles accumulated into the av groups.
  - Schedule: batch-1 projections fill the ACT-bound exp bubbles of
    batch-0 attention (per-pair kTp handoff); batch-0's out-projection
    fills batch-1 attention, which runs h2-major so batch-1's own
    out-projection can start before its last pass.
"""

import numpy as np

import concourse.bass as bass
import concourse.mybir as mybir
import concourse.tile as tile
from concourse import bacc
from concourse.bass_utils import run_bass_kernel_spmd

F32 = mybir.dt.float32
BF16 = mybir.dt.bfloat16
AF = mybir.ActivationFunctionType

NCORES = 8
B = 16
C = 512
N = 1024          # pixels = 32*32
HEADS = 8
DH = 64
NMEM = 4
PB = B // NCORES  # batch elements per core
CT = C // 128     # channel partition-tiles
NP = HEADS // 2   # head pairs
VW = HEADS * (DH + 1)  # vext width: per head [v | ones] = 65


def _build():
    nc = bacc.Bacc()
    x_ext = nc.declare_dram_parameter("x", [PB, C, N], F32, isOutput=False)
    wqkvt_ext = nc.declare_dram_parameter("wqkvt", [C, 3 * C], F32, isOutput=False)
    wot_ext = nc.declare_dram_parameter("wot", [C, C], F32, isOutput=False)
    gammat_ext = nc.declare_dram_parameter("gammat", [128, CT], F32, isOutput=False)
    memk_ext = nc.declare_dram_parameter("memk", [128, HEADS, NMEM], F32, isOutput=False)
    memv_ext = nc.declare_dram_parameter("memv", [128, 2, VW], F32, isOutput=False)
    out_ext = nc.declare_dram_parameter("out", [PB, C, N], F32, isOutput=True)

    with tile.TileContext(nc) as tc:
        with (
            tc.tile_pool(name="const", bufs=1) as const,
            tc.tile_pool(name="wstage", bufs=2) as wstage,
            tc.tile_pool(name="xp", bufs=1) as xp,
            tc.tile_pool(name="data", bufs=1) as data,
            tc.tile_pool(name="qp", bufs=2) as qp,
            tc.tile_pool(name="pp", bufs=4) as pp,
            tc.tile_pool(name="pmp", bufs=4) as pmp,
            tc.tile_pool(name="atp", bufs=2) as atp,
            tc.tile_pool(name="rp", bufs=2) as rp,
            tc.tile_pool(name="obp", bufs=4) as obp,
            tc.tile_pool(name="qkv_ps", bufs=2, space="PSUM") as qkv_ps,
            tc.tile_pool(name="sim_ps", bufs=2, space="PSUM") as sim_ps,
            tc.tile_pool(name="av_ps", bufs=1, space="PSUM") as av_ps,
        ):
            # ------------ batch-0 x load first (weights stream behind it) -------
            xraw0 = xp.tile([128, CT, N], F32, tag="xraw0")
            for t in range(CT):
                eng = nc.sync if t < 2 else nc.scalar
                eng.dma_start(out=xraw0[:, t, :], in_=x_ext[0, t * 128:(t + 1) * 128, :])

            # ---------------- per-core constants ----------------
            wqkv = const.tile([128, CT, 3 * C], BF16, tag="wqkv")
            wo = const.tile([128, CT, C], BF16, tag="wo")
            g1 = const.tile([128, CT], F32, tag="g1")
            g1q = const.tile([128, CT], F32, tag="g1q")
            ones128 = const.tile([128, 128], BF16, tag="ones128")
            kTp = const.tile([128, HEADS, N], BF16, tag="kTp")
            memk_sb = const.tile([128, HEADS, 32], BF16, tag="memk")
            vmem = const.tile([128, 2, VW], BF16, tag="vmem")
            vextA = const.tile([128, 8, VW], BF16, tag="vextA")
            vextB = const.tile([128, 8, VW], BF16, tag="vextB")

            gsb = const.tile([128, CT], F32, tag="gsb")
            nc.sync.dma_start(out=gsb, in_=gammat_ext[:, :])
            nc.scalar.activation(out=g1, in_=gsb, func=AF.Copy, bias=1.0)
            nc.scalar.activation(out=g1q, in_=gsb, func=AF.Copy, bias=1.0, scale=1.0)
            nc.scalar.mul(out=g1q, in_=g1q, mul=DH ** -0.5)

            nc.vector.memset(ones128, 1.0)
            nc.gpsimd.memset(memk_sb, 0.0)

            def weight_prep():
                for t in range(CT):
                    ws = wstage.tile([128, 3 * C], F32, tag="ws")
                    nc.sync.dma_start(out=ws, in_=wqkvt_ext[t * 128:(t + 1) * 128, :])
                    nc.vector.tensor_scalar_mul(
                        out=wqkv[:, t, 0:C], in0=ws[:, 0:C], scalar1=g1q[:, t:t + 1])
                    nc.vector.tensor_scalar_mul(
                        out=wqkv[:, t, C:3 * C], in0=ws[:, C:3 * C], scalar1=g1[:, t:t + 1])
                for t in range(CT):
                    ws = wstage.tile([128, 3 * C], F32, tag="ws")
                    nc.sync.dma_start(out=ws[:, 0:C], in_=wot_ext[t * 128:(t + 1) * 128, :])
                    nc.vector.tensor_copy(out=wo[:, t, :], in_=ws[:, 0:C])
                # mem_kv constants
                ws = wstage.tile([128, 3 * C], F32, tag="ws")
                nc.sync.dma_start(out=ws[:, 0:HEADS * NMEM],
                                  in_=memk_ext[:, :, :].rearrange("p h c -> p (h c)"))
                nc.sync.dma_start(out=ws[:, HEADS * NMEM:HEADS * NMEM + 2 * VW],
                                  in_=memv_ext[:, :, :].rearrange("p g c -> p (g c)"))
                nc.vector.tensor_copy(
                    out=memk_sb[:, :, 0:NMEM],
                    in_=ws[:, 0:HEADS * NMEM].rearrange("p (h c) -> p h c", c=NMEM))
                nc.vector.tensor_copy(
                    out=vmem,
                    in_=ws[:, HEADS * NMEM:HEADS * NMEM + 2 * VW].rearrange("p (g c) -> p g c", c=VW))
                for v in (vextA, vextB):
                    oc = v[:, :, :].rearrange("p j (h c) -> p j h c", c=DH + 1)[:, :, :, DH:DH + 1]
                    nc.gpsimd.memset(oc, 1.0)

            # ---------------- pipeline stages ----------------
            def norm(bb, xraw):
                """x -> xn (bf16, per-pixel normalized); rsqrt via exp(-ln/2)."""
                xsq = data.tile([128, CT, N], BF16, tag="xsq")
                for t in range(CT):
                    nc.vector.tensor_mul(out=xsq[:, t, :], in0=xraw[:, t, :], in1=xraw[:, t, :])
                ss = sim_ps.tile([128, N], F32, tag="sim")
                for h2 in range(2):
                    for t in range(CT):
                        nc.tensor.matmul(ss[:, h2 * 512:(h2 + 1) * 512], ones128,
                                         xsq[:, t, h2 * 512:(h2 + 1) * 512],
                                         start=(t == 0), stop=(t == CT - 1))
                lnss = data.tile([128, N], F32, tag="lnss")
                nc.scalar.activation(out=lnss, in_=ss, func=AF.Ln, scale=1.0 / C)
                snorm = data.tile([128, N], F32, tag="snorm")
                nc.scalar.activation(out=snorm, in_=lnss, func=AF.Exp, scale=-0.5)
                xn = data.tile([128, CT, N], BF16, tag="xn" + str(bb))
                for t in range(CT):
                    nc.vector.tensor_mul(out=xn[:, t, :], in0=xraw[:, t, :], in1=snorm)
                return xn

            def qkproj_group(xn, qT, mc, h2):
                """One [mc, h2] group of the q/k projection; k goes into kTp."""
                ps = qkv_ps.tile([128, 512], F32, tag="q")
                for t in range(CT):
                    nc.tensor.matmul(ps, wqkv[:, t, mc * 128:(mc + 1) * 128],
                                     xn[:, t, h2 * 512:(h2 + 1) * 512],
                                     start=(t == 0), stop=(t == CT - 1))
                if mc < 4:
                    nc.vector.tensor_copy(out=qT[:, mc, h2 * 512:(h2 + 1) * 512], in_=ps)
                else:
                    h0, h1 = 2 * (mc - 4), 2 * (mc - 4) + 1
                    nc.vector.tensor_copy(
                        out=kTp[0:64, h0, h2 * 512:(h2 + 1) * 512], in_=ps[0:64, :])
                    nc.vector.tensor_copy(
                        out=kTp[64:128, h1, h2 * 512:(h2 + 1) * 512], in_=ps[64:128, :])

            def vproj_group(xn, vext, ic):
                ps = qkv_ps.tile([128, 512], F32, tag="q")
                for t in range(CT):
                    nc.tensor.matmul(ps, xn[:, t, ic * 128:(ic + 1) * 128],
                                     wqkv[:, t, 2 * C:3 * C],
                                     start=(t == 0), stop=(t == CT - 1))
                ps_h = ps[:, :].rearrange("p (h c) -> p h c", c=DH)
                vdst = vext[:, ic, :].rearrange("p (h c) -> p h c", c=DH + 1)[:, :, 0:DH]
                nc.vector.tensor_copy(out=vdst, in_=ps_h)

            def mem_sims(qT):
                """Packed mem_kv sims: head h at psum rows 32*(h%4) of tile h//4."""
                pms = []
                for ti in range(2):
                    pmps = sim_ps.tile([128, N], F32, tag="sim")
                    for hh in range(4):
                        h = 4 * ti + hh
                        g = 32 * (h % 4)
                        r = 64 * (h % 2)
                        for h2 in range(2):
                            nc.tensor.matmul(
                                pmps[g:g + 32, h2 * 512:(h2 + 1) * 512],
                                memk_sb[r:r + 64, h, :],
                                qT[r:r + 64, h // 2, h2 * 512:(h2 + 1) * 512],
                                start=True, stop=True, tile_position=(r, g))
                    pm = pmp.tile([128, N], BF16, tag="pm")
                    nc.scalar.activation(out=pm, in_=pmps, func=AF.Exp)
                    pms.append(pm)
                return pms

            def attn_pass(P, h2, qT, vext, pms, attn, filler):
                """One (pair, h2) softmax-attention pass over 8 j-chunks + mem."""
                hA, hB = 2 * P, 2 * P + 1
                avA = av_ps.tile([65, 512], F32, tag="avA")
                avB = av_ps.tile([65, 512], F32, tag="avB")
                avs = ((hA, avA), (hB, avB))
                i0 = h2 * 512
                for c in range(8):
                    st = sim_ps.tile([128, N], F32, tag="sim")
                    nc.tensor.matmul(st[:, 0:512], kTp[0:64, hA, c * 128:(c + 1) * 128],
                                     qT[0:64, P, i0:i0 + 512], start=True, stop=True)
                    nc.tensor.matmul(st[:, 512:1024], kTp[64:128, hB, c * 128:(c + 1) * 128],
                                     qT[64:128, P, i0:i0 + 512], start=True, stop=True)
                    p = pp.tile([128, N], BF16, tag="p")
                    nc.scalar.activation(out=p, in_=st, func=AF.Exp)
                    nc.tensor.matmul(avA, vext[:, c, hA * 65:hA * 65 + 65], p[:, 0:512],
                                     start=(c == 0), stop=False)
                    nc.tensor.matmul(avB, vext[:, c, hB * 65:hB * 65 + 65], p[:, 512:1024],
                                     start=(c == 0), stop=False)
                    filler()
                for h, av in avs:
                    g = 32 * (h % 4)
                    ti = h // 4
                    c0 = (h % 4) * (DH + 1)
                    nc.tensor.matmul(av, vmem[g:g + NMEM, ti, c0:c0 + DH + 1],
                                     pms[ti][g:g + NMEM, i0:i0 + 512],
                                     start=False, stop=True, tile_position=(g, 0))
                for idx, (h, av) in enumerate(avs):
                    dr = rp.tile([1, 512], F32, tag="dr" + str(idx))
                    with tc.high_priority(offset=64):
                        nc.vector.tensor_copy(out=dr, in_=av[64:65, :])
                    rr = rp.tile([1, 512], F32, tag="rr" + str(idx))
                    nc.vector.reciprocal_approx_fast(out=rr, in_=dr)
                    rb = rp.tile([64, 512], F32, tag="rb" + str(idx))
                    nc.gpsimd.partition_broadcast(rb, rr[0:1, :])
                    r0 = 64 * (h % 2)
                    nc.vector.tensor_mul(out=attn[r0:r0 + 64, P, i0:i0 + 512],
                                         in0=av[0:64, :], in1=rb)

            def proj_group(attn, bb, mc, h2):
                ps = qkv_ps.tile([128, 512], F32, tag="q")
                for t in range(CT):
                    nc.tensor.matmul(ps, wo[:, t, mc * 128:(mc + 1) * 128],
                                     attn[:, t, h2 * 512:(h2 + 1) * 512],
                                     start=(t == 0), stop=(t == CT - 1))
                ob = obp.tile([128, 512], F32, tag="ob")
                nc.vector.tensor_copy(out=ob, in_=ps)
                nc.sync.dma_start(
                    out=out_ext[bb, mc * 128:(mc + 1) * 128, h2 * 512:(h2 + 1) * 512],
                    in_=ob)

            def make_filler(items):
                it = iter(items)

                def filler():
                    f = next(it, None)
                    if f is not None:
                        f()
                return filler, it

            # ---------------- schedule ----------------
            weight_prep()
            xn0 = norm(0, xraw0)
            xraw1 = xp.tile([128, CT, N], F32, tag="xraw1")
            for t in range(CT):
                nc.sync.dma_start(out=xraw1[:, t, :], in_=x_ext[1, t * 128:(t + 1) * 128, :])

            qT0 = qp.tile([128, NP, N], BF16, tag="qT")
            for mc in range(8):
                for h2 in range(2):
                    qkproj_group(xn0, qT0, mc, h2)
            for ic in range(8):
                vproj_group(xn0, vextA, ic)
            xn1 = norm(1, xraw1)
            pm0 = mem_sims(qT0)

            # batch-0 attention (pair-major); fillers: batch-1 projections.
            qT1 = qp.tile([128, NP, N], BF16, tag="qT")
            attn0 = atp.tile([128, CT, N], BF16, tag="attn")
            f0 = []
            for mc in range(4):
                for h2 in range(2):
                    f0.append(lambda mc=mc, h2=h2: qkproj_group(xn1, qT1, mc, h2))
            for ic in range(8):
                f0.append(lambda ic=ic: vproj_group(xn1, vextB, ic))
            # kTp handoff: k-chunk for pair Q only after pair Q's passes are done.
            f0_k = {}
            for P in range(1, 4):
                f0_k[P] = [lambda h2=h2, mc=3 + P: qkproj_group(xn1, qT1, mc, h2)
                           for h2 in range(2)]
            filler0, it0 = make_filler(f0)
            for P in range(NP):
                if P in f0_k:
                    for f in f0_k[P]:
                        f()
                for h2 in range(2):
                    attn_pass(P, h2, qT0, vextA, pm0, attn0, filler0)
            for f in it0:
                f()
            # last k-chunk (pair 3) emitted before batch-1 attention starts.
            for h2 in range(2):
                qkproj_group(xn1, qT1, 7, h2)
            pm1 = mem_sims(qT1)

            # batch-1 attention (h2-major); fillers: batch-0 out-projection,
            # then batch-1's h2=0 out-projection during the h2=1 passes.
            attn1 = atp.tile([128, CT, N], BF16, tag="attn")
            f1 = [lambda mc=mc, h2=h2: proj_group(attn0, 0, mc, h2)
                  for mc in range(4) for h2 in range(2)]
            filler1, it1 = make_filler(f1)
            for h2 in range(2):
                for P in range(NP):
                    attn_pass(P, h2, qT1, vextB, pm1, attn1, filler1)
                if h2 == 0:
                    for f in it1:
                        f()
                    f1b = [lambda mc=mc: proj_group(attn1, 1, mc, 0) for mc in range(4)]
                    filler1, it1 = make_filler(f1b)
            for f in it1:
                f()
            for mc in range(4):
                proj_group(attn1, 1, mc, 1)
    nc.compile()
    return nc


_NC_CACHE = []


def kernel(x, gamma, mem_kv, w_qkv, w_out, _trace=False):
    x = np.asarray(x, dtype=np.float32)
    gamma = np.asarray(gamma, dtype=np.float32)
    mem_kv = np.asarray(mem_kv, dtype=np.float32)
    w_qkv = np.asarray(w_qkv, dtype=np.float32)
    w_out = np.asarray(w_out, dtype=np.float32)

    b, c, hh, ww = x.shape
    n = hh * ww
    xs = x.reshape(b, c, n)

    wqkvt = np.ascontiguousarray(w_qkv.T)          # [c, 3c]
    wot = np.ascontiguousarray(w_out.T)            # [c, c]
    gammat = np.ascontiguousarray(gamma.reshape(CT, 128).T)  # [128, CT]

    memk = np.zeros((128, HEADS, NMEM), np.float32)
    memv = np.zeros((128, 2, VW), np.float32)
    for h in range(HEADS):
        r0 = 64 * (h % 2)
        memk[r0:r0 + DH, h, 0:NMEM] = mem_kv[0, h].T      # [dh, nmem]
        g, r1, c0 = h // 4, 32 * (h % 4), (h % 4) * (DH + 1)
        memv[r1:r1 + NMEM, g, c0:c0 + DH] = mem_kv[1, h]
        memv[r1:r1 + NMEM, g, c0 + DH] = 1.0

    if not _NC_CACHE:
        _NC_CACHE.append(_build())
    nc = _NC_CACHE[0]

    in_maps = []
    for core in range(NCORES):
        in_maps.append({
            "x": np.ascontiguousarray(xs[core * PB:(core + 1) * PB]),
            "wqkvt": wqkvt,
            "wot": wot,
            "gammat": gammat,
            "memk": memk,
            "memv": memv,
        })
    res = run_bass_kernel_spmd(nc, in_maps, core_ids=list(range(NCORES)), trace=_trace)
    out = np.concatenate([res.results[core]["out"] for core in range(NCORES)], axis=0)
    kernel.last_result = res
    return out.reshape(b, c, hh, ww)
